# revision 19
# baseline (speedup 1.0000x reference)
"""Trainium2 Bass kernel for nn_Attention_5420248728069.

Data-parallel over 8 NeuronCores on v_code rows; obs_code + weights
replicated; no collectives.

Math (exact refactoring of the reference):
    A   = Wq.T @ Wk                      # [E, E]
    S   = (v @ A) @ obs.T ;  s_self = rowsum((v@A) * v)
    y   = ((w0*v + expS @ obs) @ Wv.T) / Z + v    # unnormalized softmax
    out = LayerNorm(y) [* gamma + beta]

Token permutation: internal index n~ = 128*c + p <-> row n = 8*p + c
(partition-major DMA -> contiguous multi-KB descriptors; the output store
uses the same mapping so rows land canonically).

fp8 scaling (all folded, zero extra ops): A stored x4, WvT stored x8,
uT accumulated /8, recipZ = (1/(UTSCALE*WVSCALE))/Z; exp scale
1/(4*TEMP) with logit shift -2.

Schedule (per core), v2:
 - DMA priority: wq/wk quarters first across both HWDGE queues (A is
   the longest dependency chain), then v chunks 0-3, obs pairs 0/1;
   the rest paced from loop slots.
 - A matmuls per-kc as quarters land, into a 4-bank psum borrowed from
   the uT pool; vAT likewise accumulates in that pool, freeing the
   score double-buffer for the vT transposes.
 - obs transposed two chunks per matmul in fp8 DoubleRow against a
   block-diagonal 256-identity (half the PE transpose instructions,
   dense psum drains).
 - m-loop per n~-block over 32 obs chunks: scores (fp8 DR matmul) ->
   exp (ScalarE) -> Z row-sum + uT accumulation (fp8 DR matmuls).  The
   Z/uT matmuls of iteration t are issued during iteration t+1 between
   the two transpose half-groups so single-bank psum reuse never
   stalls PE.
 - epilogue: mean comes free from the residual scalar_tensor_tensor's
   accum_out; sum-of-squares via ACT Square (same table set as Exp, so
   zero table reloads); rstd via DVE bit-trick Newton.  Block-0's
   epilogue interleaves into block-1's m-loop; block-1's tail
   double-buffers its psum through the dead Z bank and stores each
   chunk as soon as it is normalized.
 - gamma/beta: the affine is compiled only when the inputs are
   non-trivial (gamma!=1 or beta!=0); the trivial variant skips those
   ops entirely.
"""

import numpy as np

N_GLOBAL = 8192
M = 4096
E = 512
CORES = 8
NLOC = N_GLOBAL // CORES  # 1024
TEMPERATURE = 22.627416997969522  # sqrt(E)
EPS = 1e-6
P = 128

NCH = NLOC // P  # 8 token chunks
MCH = M // P  # 32 obs chunks

# n~ blocks (multiples of 128; uT psum = 4*size*4B <= 8KB/partition)
BLOCKS = [512, 512]
assert sum(BLOCKS) == NLOC

ASCALE = 4.0  # A stored x4 (keeps fp8e4 out of subnormals)
WVSCALE = 8.0  # WvT stored x8
SHIFT = 2.0  # softmax logit shift: exp stays under fp8e4 max (448)
UTSCALE = 1.0 / 8.0  # uT accumulation scale
EXPSCALE = 1.0 / (ASCALE * TEMPERATURE)
RZSCALE = 1.0 / (UTSCALE * WVSCALE)  # recipZ = RZSCALE / Z

_CACHED_NC = {}


def _build(trivial_affine):
    from contextlib import ExitStack

    import concourse.tile as tile
    from concourse import bacc, mybir
    from concourse.masks import make_identity

    f32 = mybir.dt.float32
    bf16 = mybir.dt.bfloat16
    f8 = mybir.dt.float8e4
    i32 = mybir.dt.int32
    DR = mybir.MatmulPerfMode.DoubleRow
    AF = mybir.ActivationFunctionType
    ALU = mybir.AluOpType

    nc = bacc.Bacc("TRN2", target_bir_lowering=False, debug=False)

    v_d = nc.dram_tensor("v_code", [NLOC, E], f32, kind="ExternalInput")
    obs_d = nc.dram_tensor("obs_code", [M, E], f32, kind="ExternalInput")
    wq_d = nc.dram_tensor("Wq", [E, E], f32, kind="ExternalInput")
    wk_d = nc.dram_tensor("Wk", [E, E], f32, kind="ExternalInput")
    wv_d = nc.dram_tensor("Wv", [E, E], f32, kind="ExternalInput")
    gamma_d = nc.dram_tensor("gamma", [E], f32, kind="ExternalInput")
    beta_d = nc.dram_tensor("beta", [E], f32, kind="ExternalInput")
    out_d = nc.dram_tensor("out", [NLOC, E], f32, kind="ExternalOutput")

    with tile.TileContext(nc) as tc, ExitStack() as ctx:
        const = ctx.enter_context(tc.tile_pool(name="const", bufs=1))
        persist = ctx.enter_context(tc.tile_pool(name="persist", bufs=1))
        utp = ctx.enter_context(tc.tile_pool(name="utp", bufs=2))
        expp = ctx.enter_context(tc.tile_pool(name="expp", bufs=4))
        prodp = ctx.enter_context(tc.tile_pool(name="prodp", bufs=4))
        epiy = ctx.enter_context(tc.tile_pool(name="epiy", bufs=4))
        episml = ctx.enter_context(tc.tile_pool(name="episml", bufs=8))
        zsmall = ctx.enter_context(tc.tile_pool(name="zsmall", bufs=2))

        # ---------------- persistent SBUF ----------------
        v_f32 = persist.tile([P, NCH, E], f32, tag="v_f32")
        wq_f = persist.tile([P, 4, E], f32, tag="wq_f")
        wk_f = persist.tile([P, 4, E], f32, tag="wk_f")
        wv_f = persist.tile([P, 4, E], f32, tag="wv_f")
        obs_f32 = persist.tile([P, MCH, E], f32, tag="obs_f32")
        obs_f8 = persist.tile([P, MCH, E], f8, tag="obs_f8")
        obsT = persist.tile([P, 4, M], f8, tag="obsT")
        vT = persist.tile([P, 4, NLOC], f8, tag="vT")
        vAT = persist.tile([P, 4, NLOC], f8, tag="vAT")
        A_sb = persist.tile([P, 4, E], f8, tag="A")
        WvT = persist.tile([P, 4, E], f8, tag="WvT")
        wq_b = persist.tile([P, 4, E], bf16, tag="wq_b")
        wk_b = persist.tile([P, 4, E], bf16, tag="wk_b")
        w0v = persist.tile([P, 4, NLOC], bf16, tag="w0v")
        w0row = persist.tile([1, NLOC], f32, tag="w0row")
        w0bc = persist.tile([P, NLOC], f32, tag="w0bc")
        sq_scr = persist.tile([P, E], bf16, tag="sq_scr")

        identity = const.tile([P, P], f32, tag="ident")
        ident8 = const.tile([P, P], f8, tag="ident8")
        ident2 = const.tile([P, 2, 2 * P], f8, tag="ident2")
        ones_bf = const.tile([P, 1], bf16, tag="ones")
        ones_f8w = const.tile([P, 2, P], f8, tag="ones8w")
        ones_f32 = const.tile([1, 1], f32, tag="ones1")
        nshift_t = const.tile([P, 1], f32, tag="nshift")
        invE4 = const.tile([P, 4], f32, tag="invE4")
        eps4 = const.tile([P, 4], f32, tag="eps4")
        half4 = const.tile([P, 4], f32, tag="half4")
        c154 = const.tile([P, 4], f32, tag="c154")
        if not trivial_affine:
            gb_row = const.tile([1, 2, E], f32, tag="gb_row")
            gb_row_b = const.tile([1, 2, E], bf16, tag="gb_row_b")
            gamma_b = const.tile([P, E], bf16, tag="gamma")
            beta_b = const.tile([P, E], bf16, tag="beta")

        make_identity(nc, identity)
        make_identity(nc, ident8)
        # block-diagonal 256-identity for the fp8 DoubleRow pair-transpose
        nc.vector.memset(ident2, 0.0)
        nc.vector.tensor_copy(ident2[:, 0, 0:P], ident8)
        nc.vector.tensor_copy(ident2[:, 1, P : 2 * P], ident8)
        nc.vector.memset(ones_bf, 1.0)
        nc.vector.memset(ones_f8w, 1.0)
        nc.vector.memset(ones_f32, 1.0)
        nc.vector.memset(nshift_t, -SHIFT)
        nc.vector.memset(invE4, 1.0 / E)
        nc.vector.memset(eps4, EPS)
        nc.vector.memset(half4, 0.5)
        nc.vector.memset(c154, 1.5)

        out_r = out_d.ap().rearrange("(p c) e -> p c e", c=NCH)
        obs_r = obs_d.ap().rearrange("(p c) e -> p c e", c=MCH)
        v_r = v_d.ap().rearrange("(p c) e -> p c e", c=NCH)
        wq_r = wq_d.ap().rearrange("(c p) e -> p c e", p=P)
        wk_r = wk_d.ap().rearrange("(c p) e -> p c e", p=P)

        def load_obs_pair(eng, j):  # chunks 2j, 2j+1 (0.5MB)
            eng.dma_start(
                obs_f32[:, 2 * j : 2 * j + 2, :], obs_r[:, 2 * j : 2 * j + 2, :]
            )

        def load_v_quarter(eng, j):  # chunks 2j, 2j+1
            eng.dma_start(
                v_f32[:, 2 * j : 2 * j + 2, :], v_r[:, 2 * j : 2 * j + 2, :]
            )

        # ---- upfront DMA: wq/wk quarters first on both queues (the A
        # matmul chain is longest), then v chunks 0-3, then obs pair 0/1.
        nc.scalar.dma_start(wq_f[:, 0:1, :], wq_r[:, 0:1, :])
        nc.scalar.dma_start(wk_f[:, 0:1, :], wk_r[:, 0:1, :])
        nc.scalar.dma_start(wq_f[:, 1:2, :], wq_r[:, 1:2, :])
        nc.scalar.dma_start(wk_f[:, 1:2, :], wk_r[:, 1:2, :])
        load_v_quarter(nc.scalar, 0)
        load_obs_pair(nc.scalar, 0)
        load_obs_pair(nc.scalar, 2)
        load_obs_pair(nc.scalar, 4)
        nc.sync.dma_start(wq_f[:, 2:3, :], wq_r[:, 2:3, :])
        nc.sync.dma_start(wk_f[:, 2:3, :], wk_r[:, 2:3, :])
        nc.sync.dma_start(wq_f[:, 3:4, :], wq_r[:, 3:4, :])
        nc.sync.dma_start(wk_f[:, 3:4, :], wk_r[:, 3:4, :])
        load_v_quarter(nc.sync, 1)
        load_obs_pair(nc.sync, 1)
        load_obs_pair(nc.sync, 3)
        load_obs_pair(nc.sync, 5)

        # ---------------- PSUM pools (8 banks total) ----------------
        ps_s_pool = ctx.enter_context(tc.tile_pool(name="ps_s", bufs=2, space="PSUM"))
        ps_z_pool = ctx.enter_context(tc.tile_pool(name="ps_z", bufs=1, space="PSUM"))
        ps_ut_pool = ctx.enter_context(
            tc.tile_pool(name="ps_ut", bufs=1, space="PSUM")
        )
        ps_sh_pool = ctx.enter_context(
            tc.tile_pool(name="ps_sh", bufs=1, space="PSUM")
        )

        # ---------------- emission helpers ----------------
        def warmup(n):
            w = ps_sh_pool.tile([P, 256], f8, tag="sh")
            for _ in range(n):
                nc.tensor.transpose(w[:, 0:256:2], ident8, ident8)

        def cast_pair(j):
            # obs f32 -> f8 for chunks 2j, 2j+1 (ScalarE)
            nc.scalar.copy(
                obs_f8[:, 2 * j : 2 * j + 2, :], obs_f32[:, 2 * j : 2 * j + 2, :]
            )

        def tp_half(j, half, eng):
            # DR pair-transpose: obsT[:, 2h:2h+2, 2j*P:(2j+2)*P] <- chunks 2j,2j+1
            pst = ps_sh_pool.tile([P, 2, 2 * P], f32, tag="sh")
            for e in range(2):
                ec = 2 * half + e
                nc.tensor.matmul(
                    pst[:, e, :],
                    lhsT=obs_f8[:, 2 * j : 2 * j + 2, ec * P : (ec + 1) * P],
                    rhs=ident2,
                    start=True,
                    stop=True,
                    perf_mode=DR,
                )
            dst = obsT[:, 2 * half : 2 * half + 2, 2 * j * P : (2 * j + 2) * P]
            if eng is nc.scalar:
                nc.scalar.copy(dst, pst)
            else:
                eng.tensor_copy(dst, pst)

        def vt_chunk(nk, eng):
            # vT[:, :, nk*P:(nk+1)*P] <- v chunk nk (f32 transposes, f8 drain)
            pst = ps_s_pool.tile([P, 4, P], f32, tag="s")
            for ec in range(4):
                nc.tensor.transpose(
                    pst[:, ec, :], v_f32[:, nk, ec * P : (ec + 1) * P], identity
                )
            if eng is nc.scalar:
                nc.scalar.copy(vT[:, :, nk * P : (nk + 1) * P], pst)
            else:
                eng.tensor_copy(vT[:, :, nk * P : (nk + 1) * P], pst)

        def wq_cast(kc):
            nc.vector.tensor_copy(wq_b[:, kc, :], wq_f[:, kc, :])
            nc.vector.tensor_copy(wk_b[:, kc, :], wk_f[:, kc, :])

        def vat_quarter(e2, h, eng):
            # vAT[:, e2, h*512:(h+1)*512] (used for h=1 inside the m-loop)
            hs = slice(h * 512, (h + 1) * 512)
            psv = ps_s_pool.tile([P, 512], f32, tag="s")
            for u in range(2):
                nc.tensor.matmul(
                    psv,
                    lhsT=A_sb[:, 2 * u : 2 * u + 2, e2 * P : (e2 + 1) * P],
                    rhs=vT[:, 2 * u : 2 * u + 2, hs],
                    start=(u == 0),
                    stop=(u == 1),
                    perf_mode=DR,
                )
            if eng is nc.scalar:
                nc.scalar.copy(vAT[:, e2, hs], psv)
            else:
                eng.tensor_copy(vAT[:, e2, hs], psv)

        def prod_half(ec, h):
            hs = slice(h * 512, (h + 1) * 512)
            pr = prodp.tile([P, 512], bf16, tag="prod")
            nc.vector.tensor_mul(pr, vAT[:, ec, hs], vT[:, ec, hs])
            return pr

        def w0_half(h, prods):
            # self-score row -> w0row / w0bc / w0v for n~ half h
            hs = slice(h * 512, (h + 1) * 512)
            ps_sr = ps_sh_pool.tile([1, 512], f32, tag="sh")
            for ec in range(4):
                nc.tensor.matmul(
                    ps_sr,
                    lhsT=ones_bf,
                    rhs=prods[ec],
                    start=(ec == 0),
                    stop=(ec == 3),
                )
            nc.scalar.activation(
                w0row[:, hs], ps_sr, AF.Exp, bias=nshift_t[0:1, :], scale=EXPSCALE
            )
            nc.gpsimd.partition_broadcast(w0bc[:, hs], w0row[:, hs])
            for ec in range(4):
                nc.vector.scalar_tensor_tensor(
                    w0v[:, ec, hs],
                    in0=vT[:, ec, hs],
                    scalar=UTSCALE,
                    in1=w0bc[:, hs],
                    op0=ALU.mult,
                    op1=ALU.mult,
                )

        def wvt_quarter(jc):
            pst = ps_sh_pool.tile([P, 4, P], f32, tag="sh")
            for ic in range(4):
                nc.tensor.transpose(
                    pst[:, ic, :], wv_f[:, ic, jc * P : (jc + 1) * P], identity
                )
            nc.scalar.mul(WvT[:, jc, :], pst.rearrange("p a b -> p (a b)"), WVSCALE)

        uts = []
        rzs = []
        mus = {}
        sss = {}
        mu_ts = {}
        rstds = {}
        y2s = {}
        nblk = len(BLOCKS)
        starts = [sum(BLOCKS[:i]) for i in range(nblk)]

        def epi_mm(bi, k, tail=False):
            # chunk matmul + residual; mean accumulates for free, sumsq on
            # ScalarE (Square shares the Exp table: no table reloads).
            q = starts[bi] // P + k
            uTb, rzb = uts[bi], rzs[bi]
            pool = ps_sh_pool if (not tail or k % 2 == 0) else ps_z_pool
            ps_y = pool.tile(
                [P, E], f32, tag="sh" if pool is ps_sh_pool else "z", name="ps_y"
            )
            for u in range(2):
                nc.tensor.matmul(
                    ps_y,
                    lhsT=uTb[:, 2 * u : 2 * u + 2, k * P : (k + 1) * P],
                    rhs=WvT[:, 2 * u : 2 * u + 2, :],
                    start=(u == 0),
                    stop=(u == 1),
                    perf_mode=DR,
                )
            y2 = epiy.tile([P, E], bf16, tag="y2")
            nc.vector.scalar_tensor_tensor(
                y2,
                in0=ps_y,
                scalar=rzb[:, k : k + 1],
                in1=v_f32[:, q, :],
                op0=ALU.mult,
                op1=ALU.add,
                accum_out=mus[bi][:, k : k + 1],
            )
            nc.scalar.activation(
                sq_scr, y2, AF.Square, accum_out=sss[bi][:, k : k + 1]
            )
            y2s[bi].append(y2)

        def epi_rstd(bi, k0, k1, newton=2):
            # rstd = rsqrt(ss/E - (mu/E)^2 + eps) for chunks [k0, k1):
            # integer bit-trick seed on DVE (Pool has no tensor_scalar
            # opcodes), everything else on the otherwise-idle GpSimd so
            # the latency chain stays off DVE.  No Sqrt table load.
            G = nc.gpsimd
            ks = slice(k0, k1)
            if bi not in rstds:
                rstds[bi] = episml.tile([P, 4], f32, tag="rstd", name="rstd")
                mu_ts[bi] = episml.tile([P, 4], f32, tag="mu_t", name="mu_t")
            rstd = rstds[bi]
            mu_t = mu_ts[bi]
            G.tensor_mul(mu_t[:, ks], mus[bi][:, ks], invE4[:, ks])
            m2 = episml.tile([P, 4], f32, tag="m2")
            G.tensor_mul(m2[:, ks], mu_t[:, ks], mu_t[:, ks])
            ve = episml.tile([P, 4], f32, tag="ve")
            G.tensor_mul(ve[:, ks], sss[bi][:, ks], invE4[:, ks])
            G.tensor_sub(ve[:, ks], ve[:, ks], m2[:, ks])
            G.tensor_add(ve[:, ks], ve[:, ks], eps4[:, ks])
            nc.vector.tensor_scalar(
                rstd[:, ks].bitcast(i32), in0=ve[:, ks].bitcast(i32),
                scalar1=1, scalar2=None, op0=ALU.arith_shift_right,
            )
            nc.vector.tensor_scalar(
                rstd[:, ks].bitcast(i32), in0=rstd[:, ks].bitcast(i32),
                scalar1=-1, scalar2=0x5F3759DF, op0=ALU.mult, op1=ALU.add,
            )
            for _ in range(newton):
                nt = episml.tile([P, 4], f32, tag="nt")
                G.tensor_mul(nt[:, ks], rstd[:, ks], rstd[:, ks])
                G.tensor_mul(nt[:, ks], nt[:, ks], ve[:, ks])
                G.tensor_mul(nt[:, ks], nt[:, ks], half4[:, ks])
                G.tensor_sub(nt[:, ks], c154[:, ks], nt[:, ks])
                G.tensor_mul(rstd[:, ks], rstd[:, ks], nt[:, ks])

        def epi_finish(bi, k):
            q = starts[bi] // P + k
            y2 = y2s[bi][k]
            mu_t = mu_ts[bi]
            if trivial_affine:
                yout = epiy.tile([P, E], f32, tag="yout", bufs=2)
                nc.vector.tensor_scalar(
                    yout, in0=y2, scalar1=mu_t[:, k : k + 1],
                    scalar2=rstds[bi][:, k : k + 1],
                    op0=ALU.subtract, op1=ALU.mult,
                )
            else:
                nc.vector.tensor_scalar(
                    y2, in0=y2, scalar1=mu_t[:, k : k + 1],
                    scalar2=rstds[bi][:, k : k + 1],
                    op0=ALU.subtract, op1=ALU.mult,
                )
                yout = epiy.tile([P, E], f32, tag="yout", bufs=2)
                nc.vector.tensor_mul(y2, y2, gamma_b)
                nc.vector.tensor_add(yout, y2, beta_b)
            nc.sync.dma_start(out_r[:, q, :], yout)

        # ---------------- phase A ----------------
        # warmups fill the DMA lead-in so the PE clock is hot when the A
        # matmuls start (a cold p-state runs matmuls 2x slower)
        warmup(24)
        psA4 = ps_ut_pool.tile([P, 4, E], f32, tag="uT", name="psA4")
        # kc rounds ordered by DMA arrival: quarters 0,2 head both queues.
        # Warmup transposes between rounds keep the PE p-state hot across
        # the per-quarter DMA arrival gaps.
        for idx, kc in enumerate((0, 2, 1, 3)):
            wq_cast(kc)
            for ic in range(4):
                nc.tensor.matmul(
                    psA4[:, ic, :], lhsT=wq_b[:, kc, ic * P : (ic + 1) * P],
                    rhs=wk_b[:, kc, :], start=(idx == 0), stop=(idx == 3),
                )
            if idx < 3:
                warmup(10)
        # A drains: 2 on ScalarE, 2 on DVE (before the vt drains on each)
        nc.scalar.mul(A_sb[:, 0, :], psA4[:, 0, :], ASCALE)
        nc.vector.tensor_scalar(
            A_sb[:, 2, :], in0=psA4[:, 2, :], scalar1=ASCALE, scalar2=None,
            op0=ALU.mult,
        )
        nc.scalar.mul(A_sb[:, 1, :], psA4[:, 1, :], ASCALE)
        nc.vector.tensor_scalar(
            A_sb[:, 3, :], in0=psA4[:, 3, :], scalar1=ASCALE, scalar2=None,
            op0=ALU.mult,
        )
        vt_chunk(0, nc.vector)
        vt_chunk(1, nc.vector)
        vt_chunk(2, nc.scalar)
        vt_chunk(3, nc.scalar)
        cast_pair(0)
        # vAT half 0 in the 4-bank psum (freed by the A drains above);
        # u=0 needs only A chunks 0,1 + vT chunks 0,1
        psv4 = ps_ut_pool.tile([P, 4, E], f32, tag="uT", name="psv4")
        for u in range(2):
            for e2 in range(4):
                nc.tensor.matmul(
                    psv4[:, e2, :],
                    lhsT=A_sb[:, 2 * u : 2 * u + 2, e2 * P : (e2 + 1) * P],
                    rhs=vT[:, 2 * u : 2 * u + 2, 0:512],
                    start=(u == 0),
                    stop=(u == 1),
                    perf_mode=DR,
                )
        nc.vector.tensor_copy(vAT[:, 0, 0:512], psv4[:, 0, :])
        nc.scalar.copy(vAT[:, 2, 0:512], psv4[:, 2, :])
        nc.vector.tensor_copy(vAT[:, 1, 0:512], psv4[:, 1, :])
        nc.scalar.copy(vAT[:, 3, 0:512], psv4[:, 3, :])
        tp_half(0, 0, nc.vector)
        tp_half(0, 1, nc.vector)
        cast_pair(1)
        tp_half(1, 0, nc.vector)
        tp_half(1, 1, nc.vector)
        cast_pair(2)
        tp_half(2, 0, nc.vector)
        load_obs_pair(nc.scalar, 6)
        load_obs_pair(nc.sync, 7)

        # ---------------- main block loop ----------------
        prods_h = {0: [], 1: []}

        for b, (n0, bs) in enumerate(zip(starts, BLOCKS)):
            nsl = slice(n0, n0 + bs)
            nch = bs // P
            ps_uT = ps_ut_pool.tile([P, 4, bs], f32, tag="uT")
            ps_z = ps_z_pool.tile([P, bs], f32, tag="z")

            extras = {t: [] for t in range(16)}
            extras2 = {t: [] for t in range(16)}  # after z/uT of iter t-1
            if b == 0:
                # paced DMA issues
                dma_sched = {
                    0: [(nc.scalar, "obs", 8), (nc.sync, "obs", 9),
                        (nc.sync, "v", 2)],
                    1: [(nc.scalar, "obs", 10), (nc.sync, "obs", 11),
                        (nc.sync, "v", 3)],
                    2: [(nc.scalar, "obs", 12), (nc.sync, "obs", 13),
                        (nc.sync, "wv", 0)],
                    3: [(nc.scalar, "obs", 14), (nc.sync, "obs", 15)],
                }
                for t, items in dma_sched.items():
                    for eng, kind, j in items:
                        if kind == "obs":
                            extras[t].append(
                                lambda eng=eng, j=j: load_obs_pair(eng, j))
                        elif kind == "v":
                            extras[t].append(
                                lambda eng=eng, j=j: load_v_quarter(eng, j))
                        else:
                            extras[t].append(lambda: nc.sync.dma_start(
                                wv_f, wv_d.ap().rearrange("(c p) e -> p c e", p=P)))
                if not trivial_affine:
                    extras[2].append(lambda: nc.sync.dma_start(
                        gb_row[:, 0, :],
                        gamma_d.ap().rearrange("(o e) -> o e", o=1)))
                    extras[2].append(lambda: nc.sync.dma_start(
                        gb_row[:, 1, :],
                        beta_d.ap().rearrange("(o e) -> o e", o=1)))
                    extras[3].append(lambda: nc.vector.tensor_copy(
                        gb_row_b, gb_row))
                    extras[3].append(lambda: nc.gpsimd.partition_broadcast(
                        gamma_b, gb_row_b[:, 0, :]))
                    extras[3].append(lambda: nc.gpsimd.partition_broadcast(
                        beta_b, gb_row_b[:, 1, :]))
                # obs casts: pair t+3 at slot t (>=1 slot ahead of its tp)
                for t in range(13):
                    extras[t].append(lambda j=t + 3: cast_pair(j))
                # vT chunks 4-7 (v quarters 2,3 land by ~slot 2)
                for t, nk in ((2, 4), (3, 5), (4, 6), (5, 7)):
                    extras[t].append(lambda nk=nk: vt_chunk(nk, nc.vector))
                # vAT half 1 late (vT h1 ready); self-products + w0 at the end
                for t in range(10, 14):
                    extras2[t].append(
                        lambda e2=t - 10: vat_quarter(e2, 1, nc.vector))
                extras2[12].append(lambda: prods_h[0].extend(
                    [prod_half(ec, 0) for ec in range(4)]))
                extras2[13].append(lambda: prods_h[1].extend(
                    [prod_half(0, 1), prod_half(1, 1)]))
                extras2[14].append(lambda: prods_h[1].extend(
                    [prod_half(2, 1), prod_half(3, 1)]))
                extras2[14].append(lambda: w0_half(0, prods_h[0]))
                extras2[15].append(lambda: w0_half(1, prods_h[1]))
            else:
                mus[b - 1] = episml.tile([P, 4], f32, tag="mu", name="mu")
                sss[b - 1] = episml.tile([P, 4], f32, tag="ss", name="ss")
                y2s[b - 1] = []
                if b == 1:
                    for t in range(4):
                        extras[t].append(lambda jc=t: wvt_quarter(jc))
                pch = BLOCKS[b - 1] // P
                s1 = [4, 6, 8, 10]
                fin = [12, 13, 14, 15]
                for i in range(pch):
                    extras2[s1[i]].append(lambda bi=b - 1, k=i: epi_mm(bi, k))
                extras2[11].append(lambda bi=b - 1: epi_rstd(bi, 0, 4))
                for i in range(pch):
                    extras2[fin[i]].append(lambda bi=b - 1, k=i: epi_finish(bi, k))

            prev_ex = None
            for t in range(16):
                ex2 = expp.tile([P, 2, bs], f8, tag="ex")
                for j in range(2):
                    mc = 2 * t + j
                    ps_s = ps_s_pool.tile([P, bs], f32, tag="s")
                    for u in range(2):
                        nc.tensor.matmul(
                            ps_s,
                            lhsT=obsT[:, 2 * u : 2 * u + 2, mc * P : (mc + 1) * P],
                            rhs=vAT[:, 2 * u : 2 * u + 2, nsl],
                            start=(u == 0),
                            stop=(u == 1),
                            perf_mode=DR,
                        )
                    nc.scalar.activation(
                        ex2[:, j, :], ps_s, AF.Exp, bias=nshift_t, scale=EXPSCALE
                    )
                for fn in extras[t]:
                    fn()
                # second transpose half-group of pair t+2 (its first half
                # drained during the previous slot)
                if b == 0 and t <= 13:
                    tp_half(t + 2, 1, nc.vector)
                # Z/uT matmuls for the PREVIOUS iteration (exp already done)
                if prev_ex is not None:
                    tprev = t - 1
                    nc.tensor.matmul(
                        ps_z, lhsT=ones_f8w, rhs=prev_ex,
                        start=(tprev == 0), stop=False, perf_mode=DR,
                    )
                    for es in range(4):
                        nc.tensor.matmul(
                            ps_uT[:, es, :],
                            lhsT=obs_f8[
                                :, 2 * tprev : 2 * tprev + 2, es * P : (es + 1) * P
                            ],
                            rhs=prev_ex,
                            start=(tprev == 0), stop=False, perf_mode=DR,
                        )
                for fn in extras2[t]:
                    fn()
                # first half-group of pair t+3, a full slot ahead of its
                # second half so the psum-bank reuse never stalls PE
                if b == 0 and t <= 12:
                    tp_half(t + 3, 0, nc.vector)
                prev_ex = ex2
            # final iteration's Z/uT
            nc.tensor.matmul(
                ps_z, lhsT=ones_f8w, rhs=prev_ex,
                start=False, stop=True, perf_mode=DR,
            )
            for es in range(4):
                nc.tensor.matmul(
                    ps_uT[:, es, :],
                    lhsT=obs_f8[:, 30:32, es * P : (es + 1) * P],
                    rhs=prev_ex,
                    start=False, stop=True, perf_mode=DR,
                )

            # Z -> per-token recipZ first (rank-1 matmuls = on-chip
            # transpose), then drain uT (+ self term) per token chunk so
            # the tail epilogue matmul for chunk k starts right behind
            # chunk k's drain.
            uT = utp.tile([P, 4, bs], f8, tag="uTsb")
            zr = zsmall.tile([1, bs], f32, tag="zr")
            ps_zt = ps_z_pool.tile([P, nch], f32, tag="z", name="ps_zt")
            rz = zsmall.tile([P, nch], f32, tag="rz")
            nc.vector.tensor_add(zr, ps_z[0:1, :], w0row[:, nsl])
            for k in range(nch):
                nc.tensor.matmul(
                    ps_zt[:, k : k + 1],
                    lhsT=zr[:, k * P : (k + 1) * P],
                    rhs=ones_f32,
                    start=True,
                    stop=True,
                )
            nc.vector.reciprocal(rz, ps_zt)
            nc.vector.tensor_scalar(
                rz, in0=rz, scalar1=RZSCALE, scalar2=None, op0=ALU.mult
            )
            uts.append(uT)
            rzs.append(rz)
            tail = b == nblk - 1
            if tail:
                mus[b] = episml.tile([P, 4], f32, tag="mu", name="mu")
                sss[b] = episml.tile([P, 4], f32, tag="ss", name="ss")
                y2s[b] = []
            def drain_ut(k):
                nc.vector.scalar_tensor_tensor(
                    uT[:, :, k * P : (k + 1) * P],
                    in0=ps_uT[:, :, k * P : (k + 1) * P],
                    scalar=UTSCALE,
                    in1=w0v[:, :, n0 + k * P : n0 + (k + 1) * P],
                    op0=ALU.mult,
                    op1=ALU.add,
                )

            drain_ut(0)
            for k in range(nch):
                if k + 1 < nch:
                    drain_ut(k + 1)
                if tail:
                    epi_mm(b, k, tail=True)
                    if k == 1:
                        epi_rstd(b, 0, 2, newton=1)
                        epi_finish(b, 0)
                        epi_finish(b, 1)
            if tail:
                epi_rstd(b, 2, 4, newton=1)
                epi_finish(b, 2)
                epi_finish(b, 3)

    nc.compile()
    return nc


def _get_nc(trivial_affine):
    if trivial_affine not in _CACHED_NC:
        _CACHED_NC[trivial_affine] = _build(trivial_affine)
    return _CACHED_NC[trivial_affine]


def _in_maps(v_code, obs_code, Wq, Wk, Wv, gamma, beta):
    def f(x):
        return np.ascontiguousarray(np.asarray(x), dtype=np.float32)

    shared = {
        "obs_code": f(obs_code),
        "Wq": f(Wq),
        "Wk": f(Wk),
        "Wv": f(Wv),
        "gamma": f(gamma),
        "beta": f(beta),
    }
    return [
        {"v_code": f(v_code[c * NLOC : (c + 1) * NLOC]), **shared}
        for c in range(CORES)
    ]


def run(trace=False, **inputs):
    from concourse.bass_utils import run_bass_kernel_spmd

    g = np.asarray(inputs["gamma"], dtype=np.float32)
    be = np.asarray(inputs["beta"], dtype=np.float32)
    trivial = bool(np.all(g == 1.0) and np.all(be == 0.0))
    nc = _get_nc(trivial)
    res = run_bass_kernel_spmd(
        nc, _in_maps(**inputs), core_ids=list(range(CORES)), trace=trace
    )
    out = np.concatenate(
        [res.results[c]["out"] for c in range(CORES)], axis=0
    ).astype(np.float32)
    return out, res


def kernel(**inputs) -> np.ndarray:
    out, _ = run(trace=False, **inputs)
    return out
